# revision 38
# baseline (speedup 1.0000x reference)
"""Multi-head attention (B=4, S=2048, D=1024, H=16) on 8 Trainium2 cores.

Sharding: core c -> head-pair p = c (2 heads, 128 output dims), all 4
batches.  valid_len truncation is SPMD-uniform: every core runs the same
per-batch k-loop trip counts kc_b = ceil(valid_len[b]/128).  W_o is
row-split by head-pair; each core emits a full-shape [B, S, D] fp16
partial and the host sums the 8 partials.

Masking: the host zeroes xv columns at k >= valid_len[b] and supplies a
masked ones-column, so masked keys contribute exactly 0 to both the
attention*V accumulation and the softmax denominator.

Schedule (v16, ~341us vs the 451us v6 baseline):
  - Per attention step the PE-order is sc(k) -> fill work -> av(k-2):
    av runs LAGGED two steps so the in-order PE stream never waits on
    ACT's ~1.1us exp latency; projection/O-proj matmuls (queued as
    ~430-850ns atomic "fill" units) execute in between, paced
    proportionally so the queue drains evenly over remaining steps.
  - ACT runs softmax exp ONLY; all PSUM evacuation is on DVE; the
    per-qb normalize chain and the O-projection are deferred by
    attention-step counters so the slow gpsimd partition_broadcast
    never sits at the head of the in-order DVE queue; the last batch's
    first O-projection is held back as ready PE work for the drain.
  - Each dma_start of a [128,n] tile costs ~590ns of SERIAL ring-issue
    time regardless of n (128 row descriptors), so inputs use one
    full-width stream tile per (batch, kind, dj) with column-range DMAs
    (sub-tile deps gate consumers); batches run in ascending key-count
    order (smallest input appetite while the rings are cold, biggest
    batch last so its fills smooth the tail).  Outputs issue on the
    sync ring at high priority (on the scalar ring they'd block exps
    in the strict ACT FIFO).
"""

import contextlib
from collections import deque

import numpy as np
import ml_dtypes

import concourse.bacc as bacc
import concourse.mybir as mybir
import concourse.tile as tile
from concourse.bass_utils import run_bass_kernel_spmd

BF16 = mybir.dt.bfloat16
F16 = mybir.dt.float16
F32 = mybir.dt.float32
AF = mybir.ActivationFunctionType

B, S, D, H, HD = 4, 2048, 1024, 16, 64
NQB = S // 512        # query blocks of 512

_cache = {}


class _Fill:
    """Round-robin over the two single-bank fill-PSUM slots so
    consecutive fill matmul groups double-buffer across banks."""

    def __init__(self, pool):
        self.pool = pool
        self.i = 0

    def tile(self, shape, name):
        tg = "pqk" if self.i % 2 == 0 else "pv"
        self.i += 1
        return self.pool.tile(shape, F32, tag=tg, name=name)


def _emit(nc, tc, ap, kcs):
    es = contextlib.ExitStack()
    with es:
        const = es.enter_context(tc.tile_pool(name="const", bufs=1))
        resid = es.enter_context(tc.tile_pool(name="resid", bufs=1))
        stream = es.enter_context(tc.tile_pool(name="stream", bufs=8))
        expool = es.enter_context(tc.tile_pool(name="expool", bufs=3))
        wrk = es.enter_context(tc.tile_pool(name="wrk", bufs=2))

        # constants: per-dj [din-chunk, dout=128] weight tiles for the pair
        wq_sb = [const.tile([128, 128], BF16, tag=f"wq{i}", name=f"wq{i}")
                 for i in range(8)]
        wk_sb = [const.tile([128, 128], BF16, tag=f"wk{i}", name=f"wk{i}")
                 for i in range(8)]
        wv_sb = [const.tile([128, 2, HD], BF16, tag=f"wv{i}", name=f"wv{i}")
                 for i in range(8)]
        wo_sb = const.tile([128, D], BF16, tag="wo", name="wo")
        vm_sb = const.tile([128, 64, 2], BF16, tag="vmask", name="vmask")

        # residents (per batch)
        qT_sb = [resid.tile([128, S], BF16, tag=f"qT{b}", name=f"qT{b}")
                 for b in range(B)]
        kT_sb = [resid.tile([128, kcs[b] * 128], BF16, tag=f"kT{b}",
                            name=f"kT{b}") for b in range(B)]
        ctx_sb = [resid.tile([128, S], BF16, tag=f"ctx{b}", name=f"ctx{b}")
                  for b in range(B)]
        v_sb = [[resid.tile([128, 4, 2, HD + 1], BF16, tag=f"v{b}_{i}",
                            name=f"v{b}_{i}")
                 for i in range((kcs[b] + 3) // 4)]
                for b in range(B)]

        # Ascending key-count order: the first batch has the smallest
        # input appetite (the DMA rings are the scarce resource at
        # startup), and the kernel ends on the biggest batch, whose
        # projection fills smooth the tail.
        order = sorted(range(B), key=lambda b: kcs[b])

        with tc.tile_pool(name="fill_psum", bufs=1, space="PSUM") as fp, \
             tc.tile_pool(name="at_psum", bufs=1, space="PSUM") as atp:
            fill = _Fill(fp)
            filler = deque()      # (pe_cycles, closure)
            fcyc = [0]            # PE cycles currently queued in filler
            done = set()
            chunk_tiles = {}

            def fpush(cyc, fn):
                filler.append((cyc, fn))
                fcyc[0] += cyc

            def pop_one():
                cyc, fn = filler.popleft()
                fcyc[0] -= cyc
                fn()
                return cyc

            def pop_budget(budget):
                while filler and budget > 0:
                    budget -= pop_one()

            def pop_until(key):
                while key not in done:
                    pop_one()

            # One stream tile per (batch, kind, dj); DMAs land in column
            # ranges and sub-tile dependency tracking gates consumers on
            # just the ranges they read.  Each [128, n] dma_start costs
            # ~590ns of serial ring-issue time on the issuing engine
            # (128 row descriptors) REGARDLESS of n, so wide issues are
            # nearly free bandwidth and the ring is the scarce resource.

            def issue_range(eng, b, kind, dj, c0, c1):
                key = (b, kind, dj)
                if key not in chunk_tiles:
                    w = S if kind == "xq" else kcs[b] * 128
                    chunk_tiles[key] = stream.tile([128, w], BF16, tag=kind,
                                                   name=f"{kind}t")
                t = chunk_tiles[key]
                eng.dma_start(
                    t[:, c0:c1], ap[f"{kind}{b}"][dj * 128:(dj + 1) * 128,
                                                  c0:c1])

            def issue_batch_dma(b, first=False):
                """Issue a batch's input DMAs.  For the first batch the
                stream is split into 512-col range-groups alternated over
                BOTH HWDGE rings (sync + scalar) with the weight loads
                pinned to the scalar ring; later batches (prefetched a
                whole batch ahead) go full-width on sync."""
                nk = kcs[b] * 128
                if not first:
                    for kind in ("xq", "xk", "xv"):
                        w = S if kind == "xq" else nk
                        for dj in range(8):
                            issue_range(nc.sync, b, kind, dj, 0, w)
                    return
                # First batch: scalar ring carries ONLY wq+wk (so the
                # exp stream isn't stuck behind DMA issues in the Scalar
                # FIFO); sync carries the data ranges in consumption
                # order.
                load_wq()
                load_wk()
                for dj in range(8):
                    issue_range(nc.sync, b, "xq", dj, 0, 512)
                load_wv()
                for r in range(4):
                    c0, c1 = r * 512, min((r + 1) * 512, nk)
                    if c0 < nk:
                        for dj in range(8):
                            issue_range(nc.sync, b, "xk", dj, c0, c1)
                        for dj in range(8):
                            issue_range(nc.sync, b, "xv", dj, c0, c1)
                    if r > 0:
                        for dj in range(8):
                            issue_range(nc.sync, b, "xq", dj, r * 512,
                                        (r + 1) * 512)
                load_wo()

            def push_q(b, qb):
                co = qb * 512

                def half(h):
                    def fn():
                        psq = fill.tile([128, 256], "psq")
                        for dj in range(8):
                            t = chunk_tiles[(b, "xq", dj)]
                            nc.tensor.matmul(
                                psq[:],
                                wq_sb[dj][:],
                                t[:, co + h * 256:co + (h + 1) * 256],
                                start=(dj == 0), stop=(dj == 7))
                        nc.vector.tensor_copy(
                            qT_sb[b][:, qb * 512 + h * 256:
                                     qb * 512 + (h + 1) * 256], psq[:])
                        if h == 1:
                            done.add((b, 'q', qb))
                    return fn
                fpush(2048, half(0))
                fpush(2048, half(1))

            def push_k(b, blk, n):
                co = blk * 512
                halves = [(h * 256, min(256, n - h * 256))
                          for h in range(2) if n > h * 256]

                def half(idx, off, m):
                    def fn():
                        psk = fill.tile([128, 256], "psk")
                        for dj in range(8):
                            t = chunk_tiles[(b, "xk", dj)]
                            nc.tensor.matmul(
                                psk[:, 0:m], wk_sb[dj][:],
                                t[:, co + off:co + off + m],
                                start=(dj == 0), stop=(dj == 7))
                        nc.vector.tensor_copy(
                            kT_sb[b][:, blk * 512 + off:blk * 512 + off + m],
                            psk[:, 0:m])
                        if idx == len(halves) - 1:
                            done.add((b, 'k', blk))
                    return fn
                for idx, (off, m) in enumerate(halves):
                    fpush(8 * m, half(idx, off, m))

            def push_v(b, blk, n):
                co = blk * 512
                nsl = n // 128

                def slice_(sl):
                    def fn():
                        psv = fill.tile([128, 2, HD], "psv")
                        for dj in range(8):
                            t = chunk_tiles[(b, "xv", dj)]
                            nc.tensor.matmul(
                                psv[:],
                                t[:, co + sl * 128:co + (sl + 1) * 128],
                                wv_sb[dj][:], start=(dj == 0), stop=(dj == 7))
                        nc.vector.tensor_copy(
                            v_sb[b][blk][:, sl, :, 0:HD], psv[:])
                        if sl == nsl - 1:
                            # one masked-ones-column copy per 512-key block
                            s0 = b * 16 + blk * 4
                            nc.vector.tensor_copy(
                                v_sb[b][blk][:, 0:nsl, :, HD],
                                vm_sb[:, s0:s0 + nsl, :])
                            done.add((b, 'v', blk))
                    return fn
                for sl in range(nsl):
                    fpush(1024, slice_(sl))

            def push_batch_units(b):
                nk = kcs[b] * 128
                nblk = (nk + 511) // 512
                for blk in range(max(nblk, NQB)):
                    if blk < NQB:
                        push_q(b, blk)
                    if blk < nblk:
                        n = min(512, nk - blk * 512)
                        push_k(b, blk, n)
                        push_v(b, blk, n)

            def push_oproj(b, qb):
                """O-projection partial for one query block: 4 units of
                [128 s-rows x 1024 dout] each (2 matmuls + DVE evac + one
                output DMA on the scalar ring)."""
                def unit(sl):
                    def fn():
                        sc = qb * 4 + sl
                        ot = wrk.tile([128, D], F16, tag="ot", name="ot",
                                      bufs=6)
                        for ih in range(2):
                            po = fill.tile([128, 512], "po")
                            nc.tensor.matmul(
                                po[:], ctx_sb[b][:, sc * 128:(sc + 1) * 128],
                                wo_sb[:, ih * 512:(ih + 1) * 512],
                                start=True, stop=True)
                            nc.vector.tensor_copy(
                                ot[:, ih * 512:(ih + 1) * 512], po[:])
                        # high priority: jumps ahead of the queued bulk
                        # input issues on the sync ring (a blocked output
                        # would back up ot slots into the DVE pipeline)
                        with tc.high_priority():
                            nc.sync.dma_start(
                                ap["out"][b, sc * 128:(sc + 1) * 128, :],
                                ot[:])
                    return fn
                for sl in range(4):
                    fpush(1024, unit(sl))

            # ---- weight loads (interleaved with first batch's stream) --
            def load_wq():
                for i in range(8):
                    nc.scalar.dma_start(wq_sb[i][:],
                                        ap["wq"][i * 128:(i + 1) * 128, :])

            def load_wk():
                for i in range(8):
                    nc.scalar.dma_start(wk_sb[i][:],
                                        ap["wk"][i * 128:(i + 1) * 128, :])

            def load_wv():
                for i in range(8):
                    nc.sync.dma_start(wv_sb[i][:],
                                      ap["wv"][i * 128:(i + 1) * 128, :, :])
                nc.sync.dma_start(vm_sb[:], ap["vones"])

            def load_wo():
                nc.sync.dma_start(wo_sb[:], ap["wo"])

            # ---- deferred work: normalize chain + O-proj, spread over
            # later attention steps so slow cross-engine hops (gpsimd
            # broadcast) never sit at the head of the DVE/PE FIFOs.
            step_no = [0]
            deferred = []

            def defer(delta, fn):
                deferred.append([step_no[0] + delta, fn])

            def tick():
                step_no[0] += 1
                due = [x for x in deferred if x[0] <= step_no[0]]
                for x in due:
                    deferred.remove(x)
                    x[1]()

            def norm_a(avc, r0):
                def fn():
                    bc = wrk.tile([HD, 1024], F32, tag="bc", name="bc",
                                  bufs=1)
                    nc.gpsimd.partition_broadcast(bc[:], r0[0:1, :])
                    recb = wrk.tile([HD, 1024], F32, tag="recb",
                                    name="recb")
                    nc.vector.reciprocal_approx_fast(recb[:], bc[:])
                    return recb
                return fn

            def norm_b(b, qb, avc, cell):
                def fn():
                    recb = cell[0]
                    nc.gpsimd.tensor_mul(
                        ctx_sb[b][0:HD, qb * 512:(qb + 1) * 512],
                        avc[0:HD, 0, :], recb[:, 0:512])
                    tmp = wrk.tile([HD, 512], BF16, tag="tmpb", name="tmp")
                    nc.gpsimd.tensor_mul(tmp[:], avc[0:HD, 1, :],
                                         recb[:, 512:1024])
                    nc.gpsimd.dma_start(
                        ctx_sb[b][HD:128, qb * 512:(qb + 1) * 512], tmp[:])
                return fn

            # ---- main schedule ----------------------------------------
            b0 = order[0]
            issue_batch_dma(b0, first=True)
            push_batch_units(b0)

            steps_rem = [sum(NQB * kcs[b] for b in order)]
            drain_reserve = []
            for i, b in enumerate(order):
                kcb = kcs[b]
                if i + 1 < B:
                    nb = order[i + 1]
                    issue_batch_dma(nb)
                    push_batch_units(nb)
                for qb in range(NQB):
                    pop_until((b, 'q', qb))
                    av = atp.tile([HD + 1, 2, 512], F32, tag="av", name="av")
                    exs = {}

                    def do_av(t, av=av, exs=exs, b=b, kcb=kcb):
                        ex = exs.pop(t)
                        for h2 in range(2):
                            nc.tensor.matmul(
                                av[:, h2, :],
                                v_sb[b][t >> 2][:, t & 3, h2, :],
                                ex[:, h2, :],
                                start=(t == 0), stop=(t == kcb - 1))

                    for kc in range(kcb):
                        blk = kc >> 2
                        pop_until((b, 'k', blk))
                        pop_until((b, 'v', blk))
                        scp = atp.tile([128, 2, 512], F32, tag="sc",
                                       name="scp", bufs=2)
                        for h2 in range(2):
                            nc.tensor.matmul(
                                scp[:, h2, :],
                                kT_sb[b][64 * h2:64 * h2 + 64,
                                         kc * 128:(kc + 1) * 128],
                                qT_sb[b][64 * h2:64 * h2 + 64,
                                        qb * 512:(qb + 1) * 512],
                                start=True, stop=True)
                        ex = expool.tile([128, 2, 512], BF16, tag="ex",
                                         name="ex", bufs=4)
                        nc.scalar.activation(ex[:], scp[:], AF.Exp,
                                             scale=0.125)
                        exs[kc] = ex
                        tick()
                        # Fill work lands here; av runs LAGGED two steps
                        # (its ex finished long ago) so the PE stream
                        # never waits on ACT's exp latency.
                        want = fcyc[0] // max(steps_rem[0], 1)
                        pop_budget(min(max(want, 1024), 4096))
                        steps_rem[0] -= 1
                        if kc >= 2:
                            do_av(kc - 2)
                    for t in (kcb - 2, kcb - 1):
                        if t >= 0:
                            pop_budget(1024)
                            do_av(t)
                    # Evacuate av right away (frees the PSUM slot for the
                    # next qb); the divide-by-denominator chain is deferred
                    # over the next qb's steps.
                    avc = wrk.tile([HD + 1, 2, 512], F32, tag="avc",
                                   name="avc")
                    with tc.high_priority():
                        nc.vector.tensor_copy(avc[:], av[:])
                    r0 = wrk.tile([1, 1024], F32, tag="r0", name="r0",
                                  bufs=1)
                    nc.gpsimd.dma_start(r0[:], avc[HD:HD + 1, :, :])
                    cell = [None]
                    na = norm_a(avc, r0)
                    defer(1, (lambda cell=cell, na=na:
                              cell.__setitem__(0, na())))
                    defer(3, norm_b(b, qb, avc, cell))
                    if i == B - 1 and qb <= 1:
                        # hold the last batch's first O-proj back as ready
                        # PE work for the drain, so the final normalize
                        # chain doesn't leave the PE idle
                        drain_reserve.append((b, qb))
                    else:
                        defer(6, (lambda bb=b, qq=qb: push_oproj(bb, qq)))
                    pop_budget(2048)
            for bb, qq in drain_reserve:
                push_oproj(bb, qq)
            while deferred or filler:
                if deferred:
                    deferred.sort(key=lambda x: x[0])
                    deferred.pop(0)[1]()
                pop_budget(2048)
            while filler:
                pop_one()


def _build(kcs):
    key = ("nc", tuple(kcs))
    if key in _cache:
        return _cache[key]
    nc = bacc.Bacc("TRN2", target_bir_lowering=False, debug=False,
                   num_devices=8)
    ap = {"wq": nc.dram_tensor("wq", [D, 128], BF16, kind="ExternalInput").ap(),
          "wk": nc.dram_tensor("wk", [D, 128], BF16, kind="ExternalInput").ap(),
          "wv": nc.dram_tensor("wv", [D, 2, HD], BF16, kind="ExternalInput").ap(),
          "wo": nc.dram_tensor("wo", [128, D], BF16, kind="ExternalInput").ap(),
          "vones": nc.dram_tensor("vones", [128, 64, 2], BF16,
                                  kind="ExternalInput").ap(),
          "out": nc.dram_tensor("out", [B, S, D], F16,
                                kind="ExternalOutput").ap()}
    for b in range(B):
        ap[f"xq{b}"] = nc.dram_tensor(f"xq{b}", [D, S], BF16,
                                      kind="ExternalInput").ap()
        ap[f"xk{b}"] = nc.dram_tensor(f"xk{b}", [D, kcs[b] * 128], BF16,
                                      kind="ExternalInput").ap()
        ap[f"xv{b}"] = nc.dram_tensor(f"xv{b}", [D, kcs[b] * 128], BF16,
                                      kind="ExternalInput").ap()
    with tile.TileContext(nc) as tc:
        _emit(nc, tc, ap, kcs)
    nc.compile()
    _cache[key] = nc
    return nc


def _in_maps(kcs, queries, keys, values, valid_len, W_q, W_k, W_v, W_o):
    bf = ml_dtypes.bfloat16
    # host-masked ones column: 1 where k < valid_len[b], else 0
    # vones[p, b*16+sc, h] = 1 if sc*128+p < valid_len[b] else 0
    kpos = np.arange(16 * 128).reshape(16, 128)
    vones = np.zeros((128, 64, 2), bf)
    for b in range(B):
        v1 = (kpos < int(valid_len[b])).astype(bf)  # [16, 128]
        vones[:, b * 16:(b + 1) * 16, :] = v1.T[:, :, None]
    maps = []
    for c in range(8):
        j0 = 128 * c
        m = {
            "wq": np.ascontiguousarray(W_q[j0:j0 + 128, :].T).astype(bf),
            "wk": np.ascontiguousarray(W_k[j0:j0 + 128, :].T).astype(bf),
            "wv": np.ascontiguousarray(
                W_v[j0:j0 + 128, :].T).astype(bf).reshape(D, 2, HD),
            "wo": np.ascontiguousarray(W_o[:, j0:j0 + 128].T).astype(bf),
            "vones": vones,
        }
        for b in range(B):
            nk = kcs[b] * 128
            xv = values[b][:nk].T.copy()      # [D, nk]
            xv[:, int(valid_len[b]):] = 0.0   # mask padding rows of v
            m[f"xq{b}"] = np.ascontiguousarray(queries[b].T).astype(bf)
            m[f"xk{b}"] = np.ascontiguousarray(keys[b][:nk].T).astype(bf)
            m[f"xv{b}"] = xv.astype(bf)
        maps.append(m)
    return maps


def kernel(queries, keys, values, valid_len, W_q, W_k, W_v, W_o,
           _run_kwargs=None):
    queries = np.asarray(queries, np.float32)
    keys = np.asarray(keys, np.float32)
    values = np.asarray(values, np.float32)
    valid_len = np.asarray(valid_len)
    W_q = np.asarray(W_q, np.float32)
    W_k = np.asarray(W_k, np.float32)
    W_v = np.asarray(W_v, np.float32)
    W_o = np.asarray(W_o, np.float32)

    kcs = [max(1, min(16, -(-int(valid_len[b]) // 128))) for b in range(B)]
    nc = _build(kcs)
    maps = _in_maps(kcs, queries, keys, values, valid_len, W_q, W_k, W_v, W_o)
    res = run_bass_kernel_spmd(nc, maps, list(range(8)), **(_run_kwargs or {}))
    out = np.zeros((B, S, D), np.float32)
    for c in range(8):
        out += res.results[c]["out"].astype(np.float32)
    if _run_kwargs:
        _cache["last_results"] = res
    return out


# revision 40
# speedup vs baseline: 1.3504x; 1.3504x over previous
"""Multi-head attention (B=4, S=2048, D=1024, H=16) on 8 Trainium2 cores.

Sharding: core c -> head-pair p = c (2 heads, 128 output dims), all 4
batches.  valid_len truncation is SPMD-uniform: every core runs the same
per-batch k-loop trip counts kc_b = ceil(valid_len[b]/128).  W_o is
row-split by head-pair; each core emits a full-shape [B, S, D] fp16
partial and the host sums the 8 partials.

Masking: the host zeroes xv columns at k >= valid_len[b] and supplies a
masked ones-column, so masked keys contribute exactly 0 to both the
attention*V accumulation and the softmax denominator.

Schedule (v16, ~341us vs the 451us v6 baseline):
  - Per attention step the PE-order is sc(k) -> fill work -> av(k-2):
    av runs LAGGED two steps so the in-order PE stream never waits on
    ACT's ~1.1us exp latency; projection/O-proj matmuls (queued as
    ~430-850ns atomic "fill" units) execute in between, paced
    proportionally so the queue drains evenly over remaining steps.
  - ACT runs softmax exp ONLY; all PSUM evacuation is on DVE; the
    per-qb normalize chain and the O-projection are deferred by
    attention-step counters so the slow gpsimd partition_broadcast
    never sits at the head of the in-order DVE queue; the last batch's
    first O-projection is held back as ready PE work for the drain.
  - Each dma_start of a [128,n] tile costs ~590ns of SERIAL ring-issue
    time regardless of n (128 row descriptors), so inputs use one
    full-width stream tile per (batch, kind, dj) with column-range DMAs
    (sub-tile deps gate consumers); batches run in ascending key-count
    order (smallest input appetite while the rings are cold, biggest
    batch last so its fills smooth the tail).  Outputs issue on the
    sync ring at high priority (on the scalar ring they'd block exps
    in the strict ACT FIFO).
"""

import contextlib
from collections import deque

import numpy as np
import ml_dtypes

import concourse.bacc as bacc
import concourse.mybir as mybir
import concourse.tile as tile
from concourse.bass_utils import run_bass_kernel_spmd

BF16 = mybir.dt.bfloat16
F16 = mybir.dt.float16
F32 = mybir.dt.float32
AF = mybir.ActivationFunctionType

B, S, D, H, HD = 4, 2048, 1024, 16, 64
NQB = S // 512        # query blocks of 512

_cache = {}


class _Fill:
    """Round-robin over the two single-bank fill-PSUM slots so
    consecutive fill matmul groups double-buffer across banks."""

    def __init__(self, pool):
        self.pool = pool
        self.i = 0

    def tile(self, shape, name):
        tg = "pqk" if self.i % 2 == 0 else "pv"
        self.i += 1
        return self.pool.tile(shape, F32, tag=tg, name=name)


def _emit(nc, tc, ap, kcs):
    es = contextlib.ExitStack()
    with es:
        const = es.enter_context(tc.tile_pool(name="const", bufs=1))
        resid = es.enter_context(tc.tile_pool(name="resid", bufs=1))
        stream = es.enter_context(tc.tile_pool(name="stream", bufs=8))
        expool = es.enter_context(tc.tile_pool(name="expool", bufs=3))
        wrk = es.enter_context(tc.tile_pool(name="wrk", bufs=2))

        # constants: per-dj [din-chunk, dout=128] weight tiles for the pair
        wq_sb = [const.tile([128, 128], BF16, tag=f"wq{i}", name=f"wq{i}")
                 for i in range(8)]
        wk_sb = [const.tile([128, 128], BF16, tag=f"wk{i}", name=f"wk{i}")
                 for i in range(8)]
        wv_sb = [const.tile([128, 2, HD], BF16, tag=f"wv{i}", name=f"wv{i}")
                 for i in range(8)]
        wo_sb = const.tile([128, D], BF16, tag="wo", name="wo")
        vm_sb = const.tile([128, 64, 2], BF16, tag="vmask", name="vmask")

        # residents (per batch)
        qT_sb = [resid.tile([128, S], BF16, tag=f"qT{b}", name=f"qT{b}")
                 for b in range(B)]
        kT_sb = [resid.tile([128, kcs[b] * 128], BF16, tag=f"kT{b}",
                            name=f"kT{b}") for b in range(B)]
        ctx_sb = [resid.tile([128, S], BF16, tag=f"ctx{b}", name=f"ctx{b}")
                  for b in range(B)]
        v_sb = [[resid.tile([128, 4, 2, HD + 1], BF16, tag=f"v{b}_{i}",
                            name=f"v{b}_{i}")
                 for i in range((kcs[b] + 3) // 4)]
                for b in range(B)]

        # Ascending key-count order: the first batch has the smallest
        # input appetite (the DMA rings are the scarce resource at
        # startup), and the kernel ends on the biggest batch, whose
        # projection fills smooth the tail.
        order = sorted(range(B), key=lambda b: kcs[b])

        with tc.tile_pool(name="fill_psum", bufs=1, space="PSUM") as fp, \
             tc.tile_pool(name="at_psum", bufs=1, space="PSUM") as atp:
            fill = _Fill(fp)
            filler = deque()      # (pe_cycles, closure)
            fcyc = [0]            # PE cycles currently queued in filler
            done = set()
            chunk_tiles = {}

            def fpush(cyc, fn):
                filler.append((cyc, fn))
                fcyc[0] += cyc

            def pop_one():
                cyc, fn = filler.popleft()
                fcyc[0] -= cyc
                fn()
                return cyc

            def pop_budget(budget):
                while filler and budget > 0:
                    budget -= pop_one()

            def pop_until(key):
                while key not in done:
                    pop_one()

            # One stream tile per (batch, kind, dj); DMAs land in column
            # ranges and sub-tile dependency tracking gates consumers on
            # just the ranges they read.  Each [128, n] dma_start costs
            # ~590ns of serial ring-issue time on the issuing engine
            # (128 row descriptors) REGARDLESS of n, so wide issues are
            # nearly free bandwidth and the ring is the scarce resource.

            def issue_range(eng, b, kind, dj, c0, c1):
                key = (b, kind, dj)
                if key not in chunk_tiles:
                    w = S if kind == "xq" else kcs[b] * 128
                    chunk_tiles[key] = stream.tile([128, w], BF16, tag=kind,
                                                   name=f"{kind}t")
                t = chunk_tiles[key]
                eng.dma_start(
                    t[:, c0:c1], ap[f"{kind}{b}"][dj * 128:(dj + 1) * 128,
                                                  c0:c1])

            def issue_batch_dma(b, first=False):
                """Issue a batch's input DMAs.  For the first batch the
                stream is split into 512-col range-groups alternated over
                BOTH HWDGE rings (sync + scalar) with the weight loads
                pinned to the scalar ring; later batches (prefetched a
                whole batch ahead) go full-width on sync."""
                nk = kcs[b] * 128
                if not first:
                    for kind in ("xq", "xk", "xv"):
                        w = S if kind == "xq" else nk
                        for dj in range(8):
                            issue_range(nc.sync, b, kind, dj, 0, w)
                    return
                # First batch: scalar ring carries ONLY wq+wk (so the
                # exp stream isn't stuck behind DMA issues in the Scalar
                # FIFO); sync carries the data ranges in consumption
                # order.
                load_wq()
                load_wk()
                for dj in range(8):
                    issue_range(nc.sync, b, "xq", dj, 0, 512)
                load_wv()
                for r in range(4):
                    c0, c1 = r * 512, min((r + 1) * 512, nk)
                    if c0 < nk:
                        for dj in range(8):
                            issue_range(nc.sync, b, "xk", dj, c0, c1)
                        for dj in range(8):
                            issue_range(nc.sync, b, "xv", dj, c0, c1)
                    if r > 0:
                        for dj in range(8):
                            issue_range(nc.sync, b, "xq", dj, r * 512,
                                        (r + 1) * 512)
                load_wo()

            def push_q(b, qb):
                co = qb * 512

                def half(h):
                    def fn():
                        psq = fill.tile([128, 256], "psq")
                        for dj in range(8):
                            t = chunk_tiles[(b, "xq", dj)]
                            nc.tensor.matmul(
                                psq[:],
                                wq_sb[dj][:],
                                t[:, co + h * 256:co + (h + 1) * 256],
                                start=(dj == 0), stop=(dj == 7))
                        nc.vector.tensor_copy(
                            qT_sb[b][:, qb * 512 + h * 256:
                                     qb * 512 + (h + 1) * 256], psq[:])
                        if h == 1:
                            done.add((b, 'q', qb))
                    return fn
                fpush(2048, half(0))
                fpush(2048, half(1))

            def push_k(b, blk, n):
                co = blk * 512
                halves = [(h * 256, min(256, n - h * 256))
                          for h in range(2) if n > h * 256]

                def half(idx, off, m):
                    def fn():
                        psk = fill.tile([128, 256], "psk")
                        for dj in range(8):
                            t = chunk_tiles[(b, "xk", dj)]
                            nc.tensor.matmul(
                                psk[:, 0:m], wk_sb[dj][:],
                                t[:, co + off:co + off + m],
                                start=(dj == 0), stop=(dj == 7))
                        nc.vector.tensor_copy(
                            kT_sb[b][:, blk * 512 + off:blk * 512 + off + m],
                            psk[:, 0:m])
                        if idx == len(halves) - 1:
                            done.add((b, 'k', blk))
                    return fn
                for idx, (off, m) in enumerate(halves):
                    fpush(8 * m, half(idx, off, m))

            def push_v(b, blk, n):
                co = blk * 512
                nsl = n // 128

                def slice_(sl):
                    def fn():
                        psv = fill.tile([128, 2, HD], "psv")
                        for dj in range(8):
                            t = chunk_tiles[(b, "xv", dj)]
                            nc.tensor.matmul(
                                psv[:],
                                t[:, co + sl * 128:co + (sl + 1) * 128],
                                wv_sb[dj][:], start=(dj == 0), stop=(dj == 7))
                        nc.vector.tensor_copy(
                            v_sb[b][blk][:, sl, :, 0:HD], psv[:])
                        if sl == nsl - 1:
                            # one masked-ones-column copy per 512-key block
                            s0 = b * 16 + blk * 4
                            nc.vector.tensor_copy(
                                v_sb[b][blk][:, 0:nsl, :, HD],
                                vm_sb[:, s0:s0 + nsl, :])
                            done.add((b, 'v', blk))
                    return fn
                for sl in range(nsl):
                    fpush(1024, slice_(sl))

            def push_batch_units(b):
                nk = kcs[b] * 128
                nblk = (nk + 511) // 512
                for blk in range(max(nblk, NQB)):
                    if blk < NQB:
                        push_q(b, blk)
                    if blk < nblk:
                        n = min(512, nk - blk * 512)
                        push_k(b, blk, n)
                        push_v(b, blk, n)

            def push_oproj(b, qb):
                """O-projection partial for one query block: 4 units of
                [128 s-rows x 1024 dout] each (2 matmuls + DVE evac + one
                output DMA on the scalar ring)."""
                def unit(sl):
                    def fn():
                        sc = qb * 4 + sl
                        ot = wrk.tile([128, D], F16, tag="ot", name="ot",
                                      bufs=6)
                        for ih in range(2):
                            po = fill.tile([128, 512], "po")
                            nc.tensor.matmul(
                                po[:], ctx_sb[b][:, sc * 128:(sc + 1) * 128],
                                wo_sb[:, ih * 512:(ih + 1) * 512],
                                start=True, stop=True)
                            nc.vector.tensor_copy(
                                ot[:, ih * 512:(ih + 1) * 512], po[:])
                        # high priority: jumps ahead of the queued bulk
                        # input issues on the sync ring (a blocked output
                        # would back up ot slots into the DVE pipeline)
                        with tc.high_priority():
                            nc.sync.dma_start(
                                ap["out"][b, sc * 128:(sc + 1) * 128, :],
                                ot[:])
                    return fn
                for sl in range(4):
                    fpush(1024, unit(sl))

            # ---- weight loads (interleaved with first batch's stream) --
            def load_wq():
                for i in range(8):
                    nc.scalar.dma_start(wq_sb[i][:],
                                        ap["wq"][i * 128:(i + 1) * 128, :])

            def load_wk():
                for i in range(8):
                    nc.scalar.dma_start(wk_sb[i][:],
                                        ap["wk"][i * 128:(i + 1) * 128, :])

            def load_wv():
                for i in range(8):
                    nc.sync.dma_start(wv_sb[i][:],
                                      ap["wv"][i * 128:(i + 1) * 128, :, :])
                nc.sync.dma_start(vm_sb[:], ap["vones"])

            def load_wo():
                nc.sync.dma_start(wo_sb[:], ap["wo"])

            # ---- deferred work: normalize chain + O-proj, spread over
            # later attention steps so slow cross-engine hops (gpsimd
            # broadcast) never sit at the head of the DVE/PE FIFOs.
            step_no = [0]
            deferred = []

            def defer(delta, fn):
                deferred.append([step_no[0] + delta, fn])

            def tick():
                step_no[0] += 1
                due = [x for x in deferred if x[0] <= step_no[0]]
                for x in due:
                    deferred.remove(x)
                    x[1]()

            def norm_a(avc, r0):
                def fn():
                    bc = wrk.tile([HD, 1024], F32, tag="bc", name="bc",
                                  bufs=1)
                    nc.gpsimd.partition_broadcast(bc[:], r0[0:1, :])
                    recb = wrk.tile([HD, 1024], F32, tag="recb",
                                    name="recb")
                    nc.vector.reciprocal_approx_fast(recb[:], bc[:])
                    return recb
                return fn

            def norm_b(b, qb, avc, cell):
                def fn():
                    recb = cell[0]
                    nc.vector.tensor_mul(
                        ctx_sb[b][0:HD, qb * 512:(qb + 1) * 512],
                        avc[0:HD, 0, :], recb[:, 0:512])
                    tmp = wrk.tile([HD, 512], BF16, tag="tmpb", name="tmp")
                    nc.vector.tensor_mul(tmp[:], avc[0:HD, 1, :],
                                         recb[:, 512:1024])
                    nc.gpsimd.dma_start(
                        ctx_sb[b][HD:128, qb * 512:(qb + 1) * 512], tmp[:])
                return fn

            # ---- main schedule ----------------------------------------
            b0 = order[0]
            issue_batch_dma(b0, first=True)
            push_batch_units(b0)

            steps_rem = [sum(NQB * kcs[b] for b in order)]
            drain_reserve = []
            for i, b in enumerate(order):
                kcb = kcs[b]
                if i + 1 < B:
                    nb = order[i + 1]
                    issue_batch_dma(nb)
                    push_batch_units(nb)
                for qb in range(NQB):
                    pop_until((b, 'q', qb))
                    av = atp.tile([HD + 1, 2, 512], F32, tag="av", name="av")
                    exs = {}

                    def do_av(t, av=av, exs=exs, b=b, kcb=kcb):
                        ex = exs.pop(t)
                        for h2 in range(2):
                            nc.tensor.matmul(
                                av[:, h2, :],
                                v_sb[b][t >> 2][:, t & 3, h2, :],
                                ex[:, h2, :],
                                start=(t == 0), stop=(t == kcb - 1))

                    for kc in range(kcb):
                        blk = kc >> 2
                        pop_until((b, 'k', blk))
                        pop_until((b, 'v', blk))
                        scp = atp.tile([128, 2, 512], F32, tag="sc",
                                       name="scp", bufs=2)
                        for h2 in range(2):
                            nc.tensor.matmul(
                                scp[:, h2, :],
                                kT_sb[b][64 * h2:64 * h2 + 64,
                                         kc * 128:(kc + 1) * 128],
                                qT_sb[b][64 * h2:64 * h2 + 64,
                                        qb * 512:(qb + 1) * 512],
                                start=True, stop=True)
                        ex = expool.tile([128, 2, 512], BF16, tag="ex",
                                         name="ex", bufs=4)
                        nc.scalar.activation(ex[:], scp[:], AF.Exp,
                                             scale=0.125)
                        exs[kc] = ex
                        tick()
                        # Fill work lands here; av runs LAGGED two steps
                        # (its ex finished long ago) so the PE stream
                        # never waits on ACT's exp latency.
                        want = fcyc[0] // max(steps_rem[0], 1)
                        pop_budget(min(max(want, 1024), 4096))
                        steps_rem[0] -= 1
                        if kc >= 2:
                            do_av(kc - 2)
                    for t in (kcb - 2, kcb - 1):
                        if t >= 0:
                            pop_budget(1024)
                            do_av(t)
                    # Evacuate av right away (frees the PSUM slot for the
                    # next qb); the divide-by-denominator chain is deferred
                    # over the next qb's steps.
                    avc = wrk.tile([HD + 1, 2, 512], F32, tag="avc",
                                   name="avc")
                    with tc.high_priority():
                        nc.vector.tensor_copy(avc[:], av[:])
                    r0 = wrk.tile([1, 1024], F32, tag="r0", name="r0",
                                  bufs=1)
                    nc.gpsimd.dma_start(r0[:], avc[HD:HD + 1, :, :])
                    cell = [None]
                    na = norm_a(avc, r0)
                    defer(1, (lambda cell=cell, na=na:
                              cell.__setitem__(0, na())))
                    defer(3, norm_b(b, qb, avc, cell))
                    if i == B - 1 and qb <= 1:
                        # hold the last batch's first O-proj back as ready
                        # PE work for the drain, so the final normalize
                        # chain doesn't leave the PE idle
                        drain_reserve.append((b, qb))
                    else:
                        defer(6, (lambda bb=b, qq=qb: push_oproj(bb, qq)))
                    pop_budget(2048)
            for bb, qq in drain_reserve:
                push_oproj(bb, qq)
            while deferred or filler:
                if deferred:
                    deferred.sort(key=lambda x: x[0])
                    deferred.pop(0)[1]()
                pop_budget(2048)
            while filler:
                pop_one()


def _build(kcs):
    key = ("nc", tuple(kcs))
    if key in _cache:
        return _cache[key]
    nc = bacc.Bacc("TRN2", target_bir_lowering=False, debug=False,
                   num_devices=8)
    ap = {"wq": nc.dram_tensor("wq", [D, 128], BF16, kind="ExternalInput").ap(),
          "wk": nc.dram_tensor("wk", [D, 128], BF16, kind="ExternalInput").ap(),
          "wv": nc.dram_tensor("wv", [D, 2, HD], BF16, kind="ExternalInput").ap(),
          "wo": nc.dram_tensor("wo", [128, D], BF16, kind="ExternalInput").ap(),
          "vones": nc.dram_tensor("vones", [128, 64, 2], BF16,
                                  kind="ExternalInput").ap(),
          "out": nc.dram_tensor("out", [B, S, D], F16,
                                kind="ExternalOutput").ap()}
    for b in range(B):
        ap[f"xq{b}"] = nc.dram_tensor(f"xq{b}", [D, S], BF16,
                                      kind="ExternalInput").ap()
        ap[f"xk{b}"] = nc.dram_tensor(f"xk{b}", [D, kcs[b] * 128], BF16,
                                      kind="ExternalInput").ap()
        ap[f"xv{b}"] = nc.dram_tensor(f"xv{b}", [D, kcs[b] * 128], BF16,
                                      kind="ExternalInput").ap()
    with tile.TileContext(nc) as tc:
        _emit(nc, tc, ap, kcs)
    nc.compile()
    _cache[key] = nc
    return nc


def _in_maps(kcs, queries, keys, values, valid_len, W_q, W_k, W_v, W_o):
    bf = ml_dtypes.bfloat16
    # host-masked ones column: 1 where k < valid_len[b], else 0
    # vones[p, b*16+sc, h] = 1 if sc*128+p < valid_len[b] else 0
    kpos = np.arange(16 * 128).reshape(16, 128)
    vones = np.zeros((128, 64, 2), bf)
    for b in range(B):
        v1 = (kpos < int(valid_len[b])).astype(bf)  # [16, 128]
        vones[:, b * 16:(b + 1) * 16, :] = v1.T[:, :, None]
    maps = []
    for c in range(8):
        j0 = 128 * c
        m = {
            "wq": np.ascontiguousarray(W_q[j0:j0 + 128, :].T).astype(bf),
            "wk": np.ascontiguousarray(W_k[j0:j0 + 128, :].T).astype(bf),
            "wv": np.ascontiguousarray(
                W_v[j0:j0 + 128, :].T).astype(bf).reshape(D, 2, HD),
            "wo": np.ascontiguousarray(W_o[:, j0:j0 + 128].T).astype(bf),
            "vones": vones,
        }
        for b in range(B):
            nk = kcs[b] * 128
            xv = values[b][:nk].T.copy()      # [D, nk]
            xv[:, int(valid_len[b]):] = 0.0   # mask padding rows of v
            m[f"xq{b}"] = np.ascontiguousarray(queries[b].T).astype(bf)
            m[f"xk{b}"] = np.ascontiguousarray(keys[b][:nk].T).astype(bf)
            m[f"xv{b}"] = xv.astype(bf)
        maps.append(m)
    return maps


def kernel(queries, keys, values, valid_len, W_q, W_k, W_v, W_o,
           _run_kwargs=None):
    queries = np.asarray(queries, np.float32)
    keys = np.asarray(keys, np.float32)
    values = np.asarray(values, np.float32)
    valid_len = np.asarray(valid_len)
    W_q = np.asarray(W_q, np.float32)
    W_k = np.asarray(W_k, np.float32)
    W_v = np.asarray(W_v, np.float32)
    W_o = np.asarray(W_o, np.float32)

    kcs = [max(1, min(16, -(-int(valid_len[b]) // 128))) for b in range(B)]
    nc = _build(kcs)
    maps = _in_maps(kcs, queries, keys, values, valid_len, W_q, W_k, W_v, W_o)
    res = run_bass_kernel_spmd(nc, maps, list(range(8)), **(_run_kwargs or {}))
    out = np.zeros((B, S, D), np.float32)
    for c in range(8):
        out += res.results[c]["out"].astype(np.float32)
    if _run_kwargs:
        _cache["last_results"] = res
    return out


# revision 44
# speedup vs baseline: 1.3659x; 1.0115x over previous
"""Multi-head attention (B=4, S=2048, D=1024, H=16) on 8 Trainium2 cores.

Sharding: core c -> head-pair p = c (2 heads, 128 output dims), all 4
batches.  valid_len truncation is SPMD-uniform: every core runs the same
per-batch k-loop trip counts kc_b = ceil(valid_len[b]/128).  W_o is
row-split by head-pair; each core emits a full-shape [B, S, D] fp16
partial and the host sums the 8 partials.

Masking: the host zeroes xv columns at k >= valid_len[b] and supplies a
masked ones-column, so masked keys contribute exactly 0 to both the
attention*V accumulation and the softmax denominator.

v7 (over v6, which ran at ~405-450 us):
  - Per attention step the PE-order is sc(k) -> fill work -> av(k), so
    the projection/O-proj matmuls execute while ACT runs exp(k); v6
    emitted av before the fills, so the in-order PE stream idled on the
    exp semaphore every step (~45% PE idle + HAM clock droop to 1.2GHz).
  - Fill work is queued as ~430-850ns atomic units (Q/K projections in
    column halves, V per 128-key slice, O-proj per 128-row slice) and
    popped against a per-step cycle budget; data-dependency guards pop
    the current batch's own units just-in-time, which also removes the
    serial prefill phase at kernel start.
  - ACT runs softmax exp ONLY.  O-proj PSUM evacuation moved to DVE;
    output DMA issue moved to the scalar HWDGE ring (each [128,n]
    dma_start costs ~590ns of ring issue regardless of n, and the sync
    ring was 72% busy).
  - Input DMA in [128,1024] chunks (4x wider than v6): sync-ring issue
    cost drops from ~280us to ~110us.  Whole batches are prefetched one
    batch ahead; weight loads are interleaved with the first batch's
    chunks in consumption order.
"""

import contextlib
from collections import deque

import numpy as np
import ml_dtypes

import concourse.bacc as bacc
import concourse.mybir as mybir
import concourse.tile as tile
from concourse.bass_utils import run_bass_kernel_spmd

BF16 = mybir.dt.bfloat16
F16 = mybir.dt.float16
F32 = mybir.dt.float32
AF = mybir.ActivationFunctionType

B, S, D, H, HD = 4, 2048, 1024, 16, 64
NQB = S // 512        # query blocks of 512

_cache = {}


class _Fill:
    """Round-robin over the two single-bank fill-PSUM slots so
    consecutive fill matmul groups double-buffer across banks."""

    def __init__(self, pool):
        self.pool = pool
        self.i = 0

    def tile(self, shape, name):
        tg = "pqk" if self.i % 2 == 0 else "pv"
        self.i += 1
        return self.pool.tile(shape, F32, tag=tg, name=name)


def _emit(nc, tc, ap, kcs):
    es = contextlib.ExitStack()
    with es:
        const = es.enter_context(tc.tile_pool(name="const", bufs=1))
        resid = es.enter_context(tc.tile_pool(name="resid", bufs=1))
        stream = es.enter_context(tc.tile_pool(name="stream", bufs=8))
        expool = es.enter_context(tc.tile_pool(name="expool", bufs=3))
        wrk = es.enter_context(tc.tile_pool(name="wrk", bufs=2))

        # constants: per-dj [din-chunk, dout=128] weight tiles for the pair
        wq_sb = [const.tile([128, 128], BF16, tag=f"wq{i}", name=f"wq{i}")
                 for i in range(8)]
        wk_sb = [const.tile([128, 128], BF16, tag=f"wk{i}", name=f"wk{i}")
                 for i in range(8)]
        wv_sb = [const.tile([128, 2, HD], BF16, tag=f"wv{i}", name=f"wv{i}")
                 for i in range(8)]
        wo_sb = const.tile([128, D], BF16, tag="wo", name="wo")
        vm_sb = const.tile([128, 64, 2], BF16, tag="vmask", name="vmask")

        # residents (per batch)
        qT_sb = [resid.tile([128, S], BF16, tag=f"qT{b}", name=f"qT{b}")
                 for b in range(B)]
        kT_sb = [resid.tile([128, kcs[b] * 128], BF16, tag=f"kT{b}",
                            name=f"kT{b}") for b in range(B)]
        ctx_sb = [resid.tile([128, S], BF16, tag=f"ctx{b}", name=f"ctx{b}")
                  for b in range(B)]
        v_sb = [[resid.tile([128, 4, 2, HD + 1], BF16, tag=f"v{b}_{i}",
                            name=f"v{b}_{i}")
                 for i in range((kcs[b] + 3) // 4)]
                for b in range(B)]

        # Ascending key-count order: the first batch has the smallest
        # input appetite (the DMA rings are the scarce resource at
        # startup), and the kernel ends on the biggest batch, whose
        # projection fills smooth the tail.
        order = sorted(range(B), key=lambda b: kcs[b])

        with tc.tile_pool(name="fill_psum", bufs=1, space="PSUM") as fp, \
             tc.tile_pool(name="at_psum", bufs=1, space="PSUM") as atp:
            fill = _Fill(fp)
            filler = deque()      # (pe_cycles, closure)
            fcyc = [0]            # PE cycles currently queued in filler
            done = set()
            chunk_tiles = {}

            def fpush(cyc, fn):
                filler.append((cyc, fn))
                fcyc[0] += cyc

            def pop_one():
                cyc, fn = filler.popleft()
                fcyc[0] -= cyc
                fn()
                return cyc

            def pop_budget(budget):
                while filler and budget > 0:
                    budget -= pop_one()

            def pop_until(key):
                while key not in done:
                    pop_one()

            # One stream tile per (batch, kind, dj); DMAs land in column
            # ranges and sub-tile dependency tracking gates consumers on
            # just the ranges they read.  Each [128, n] dma_start costs
            # ~590ns of serial ring-issue time on the issuing engine
            # (128 row descriptors) REGARDLESS of n, so wide issues are
            # nearly free bandwidth and the ring is the scarce resource.

            def issue_range(eng, b, kind, dj, c0, c1):
                key = (b, kind, dj)
                if key not in chunk_tiles:
                    w = S if kind == "xq" else kcs[b] * 128
                    chunk_tiles[key] = stream.tile([128, w], BF16, tag=kind,
                                                   name=f"{kind}t")
                t = chunk_tiles[key]
                eng.dma_start(
                    t[:, c0:c1], ap[f"{kind}{b}"][dj * 128:(dj + 1) * 128,
                                                  c0:c1])

            def issue_batch_dma(b, first=False):
                """Issue a batch's input DMAs.  For the first batch the
                stream is split into 512-col range-groups alternated over
                BOTH HWDGE rings (sync + scalar) with the weight loads
                pinned to the scalar ring; later batches (prefetched a
                whole batch ahead) go full-width on sync."""
                nk = kcs[b] * 128
                if not first:
                    for kind in ("xq", "xk", "xv"):
                        w = S if kind == "xq" else nk
                        for dj in range(8):
                            issue_range(nc.sync, b, kind, dj, 0, w)
                    return
                # First batch: scalar ring carries ONLY wq+wk (so the
                # exp stream isn't stuck behind DMA issues in the Scalar
                # FIFO); sync carries the data ranges in consumption
                # order.
                load_wq()
                load_wk()
                for dj in range(8):
                    issue_range(nc.sync, b, "xq", dj, 0, 512)
                load_wv()
                for r in range(4):
                    c0, c1 = r * 512, min((r + 1) * 512, nk)
                    if c0 < nk:
                        for dj in range(8):
                            issue_range(nc.sync, b, "xk", dj, c0, c1)
                        for dj in range(8):
                            issue_range(nc.sync, b, "xv", dj, c0, c1)
                    if r > 0:
                        for dj in range(8):
                            issue_range(nc.sync, b, "xq", dj, r * 512,
                                        (r + 1) * 512)
                load_wo()

            def push_q(b, qb):
                co = qb * 512

                def half(h):
                    def fn():
                        psq = fill.tile([128, 256], "psq")
                        for dj in range(8):
                            t = chunk_tiles[(b, "xq", dj)]
                            nc.tensor.matmul(
                                psq[:],
                                wq_sb[dj][:],
                                t[:, co + h * 256:co + (h + 1) * 256],
                                start=(dj == 0), stop=(dj == 7))
                        nc.vector.tensor_copy(
                            qT_sb[b][:, qb * 512 + h * 256:
                                     qb * 512 + (h + 1) * 256], psq[:])
                        if h == 1:
                            done.add((b, 'q', qb))
                    return fn
                fpush(2048, half(0))
                fpush(2048, half(1))

            def push_k(b, blk, n):
                co = blk * 512
                halves = [(h * 256, min(256, n - h * 256))
                          for h in range(2) if n > h * 256]

                def half(idx, off, m):
                    def fn():
                        psk = fill.tile([128, 256], "psk")
                        for dj in range(8):
                            t = chunk_tiles[(b, "xk", dj)]
                            nc.tensor.matmul(
                                psk[:, 0:m], wk_sb[dj][:],
                                t[:, co + off:co + off + m],
                                start=(dj == 0), stop=(dj == 7))
                        nc.vector.tensor_copy(
                            kT_sb[b][:, blk * 512 + off:blk * 512 + off + m],
                            psk[:, 0:m])
                        if idx == len(halves) - 1:
                            done.add((b, 'k', blk))
                    return fn
                for idx, (off, m) in enumerate(halves):
                    fpush(8 * m, half(idx, off, m))

            def push_v(b, blk, n):
                co = blk * 512
                nsl = n // 128

                def slice_(sl):
                    def fn():
                        psv = fill.tile([128, 2, HD], "psv")
                        for dj in range(8):
                            t = chunk_tiles[(b, "xv", dj)]
                            nc.tensor.matmul(
                                psv[:],
                                t[:, co + sl * 128:co + (sl + 1) * 128],
                                wv_sb[dj][:], start=(dj == 0), stop=(dj == 7))
                        nc.vector.tensor_copy(
                            v_sb[b][blk][:, sl, :, 0:HD], psv[:])
                        if sl == nsl - 1:
                            # one masked-ones-column copy per 512-key block
                            s0 = b * 16 + blk * 4
                            nc.vector.tensor_copy(
                                v_sb[b][blk][:, 0:nsl, :, HD],
                                vm_sb[:, s0:s0 + nsl, :])
                            done.add((b, 'v', blk))
                    return fn
                for sl in range(nsl):
                    fpush(1024, slice_(sl))

            def push_batch_units(b):
                nk = kcs[b] * 128
                nblk = (nk + 511) // 512
                for blk in range(max(nblk, NQB)):
                    if blk < NQB:
                        push_q(b, blk)
                    if blk < nblk:
                        n = min(512, nk - blk * 512)
                        push_k(b, blk, n)
                        push_v(b, blk, n)

            def push_oproj(b, qb):
                """O-projection partial for one query block: 4 units of
                [128 s-rows x 1024 dout] each (2 matmuls + DVE evac + one
                output DMA on the scalar ring)."""
                def unit(sl):
                    def fn():
                        sc = qb * 4 + sl
                        ot = wrk.tile([128, D], F16, tag="ot", name="ot",
                                      bufs=6)
                        for ih in range(2):
                            po = fill.tile([128, 512], "po")
                            nc.tensor.matmul(
                                po[:], ctx_sb[b][:, sc * 128:(sc + 1) * 128],
                                wo_sb[:, ih * 512:(ih + 1) * 512],
                                start=True, stop=True)
                            nc.vector.tensor_copy(
                                ot[:, ih * 512:(ih + 1) * 512], po[:])
                        # high priority: jumps ahead of the queued bulk
                        # input issues on the sync ring (a blocked output
                        # would back up ot slots into the DVE pipeline)
                        with tc.high_priority():
                            nc.sync.dma_start(
                                ap["out"][b, sc * 128:(sc + 1) * 128, :],
                                ot[:])
                    return fn
                for sl in range(4):
                    fpush(1024, unit(sl))

            # ---- weight loads (interleaved with first batch's stream) --
            def load_wq():
                for i in range(8):
                    nc.scalar.dma_start(wq_sb[i][:],
                                        ap["wq"][i * 128:(i + 1) * 128, :])

            def load_wk():
                for i in range(8):
                    nc.scalar.dma_start(wk_sb[i][:],
                                        ap["wk"][i * 128:(i + 1) * 128, :])

            def load_wv():
                for i in range(8):
                    nc.sync.dma_start(wv_sb[i][:],
                                      ap["wv"][i * 128:(i + 1) * 128, :, :])
                nc.sync.dma_start(vm_sb[:], ap["vones"])

            def load_wo():
                nc.sync.dma_start(wo_sb[:], ap["wo"])

            # ---- deferred work: normalize chain + O-proj, spread over
            # later attention steps so slow cross-engine hops (gpsimd
            # broadcast) never sit at the head of the DVE/PE FIFOs.
            step_no = [0]
            deferred = []

            def defer(delta, fn):
                deferred.append([step_no[0] + delta, fn])

            def tick():
                step_no[0] += 1
                due = [x for x in deferred if x[0] <= step_no[0]]
                for x in due:
                    deferred.remove(x)
                    x[1]()

            def norm_a(avc, r0):
                def fn():
                    bc = wrk.tile([HD, 1024], F32, tag="bc", name="bc",
                                  bufs=1)
                    nc.gpsimd.partition_broadcast(bc[:], r0[0:1, :])
                    recb = wrk.tile([HD, 1024], F32, tag="recb",
                                    name="recb")
                    nc.vector.reciprocal_approx_fast(recb[:], bc[:])
                    return recb
                return fn

            def norm_b(b, qb, avc, cell):
                def fn():
                    recb = cell[0]
                    nc.vector.tensor_mul(
                        ctx_sb[b][0:HD, qb * 512:(qb + 1) * 512],
                        avc[0:HD, 0, :], recb[:, 0:512])
                    tmp = wrk.tile([HD, 512], BF16, tag="tmpb", name="tmp")
                    nc.vector.tensor_mul(tmp[:], avc[0:HD, 1, :],
                                         recb[:, 512:1024])
                    nc.gpsimd.dma_start(
                        ctx_sb[b][HD:128, qb * 512:(qb + 1) * 512], tmp[:])
                return fn

            # ---- main schedule ----------------------------------------
            b0 = order[0]
            issue_batch_dma(b0, first=True)
            push_batch_units(b0)

            steps_rem = [sum(NQB * kcs[b] for b in order)]
            drain_reserve = []
            for i, b in enumerate(order):
                kcb = kcs[b]
                if i + 1 < B:
                    nb = order[i + 1]
                    issue_batch_dma(nb)
                    push_batch_units(nb)
                for qb in range(NQB):
                    pop_until((b, 'q', qb))
                    av = atp.tile([HD + 1, 2, 512], F32, tag="av", name="av")
                    exs = {}

                    def do_av(t, av=av, exs=exs, b=b, kcb=kcb):
                        ex = exs.pop(t)
                        for h2 in range(2):
                            nc.tensor.matmul(
                                av[:, h2, :],
                                v_sb[b][t >> 2][:, t & 3, h2, :],
                                ex[:, h2, :],
                                start=(t == 0), stop=(t == kcb - 1))

                    for kc in range(kcb):
                        blk = kc >> 2
                        pop_until((b, 'k', blk))
                        pop_until((b, 'v', blk))
                        scp = atp.tile([128, 2, 512], F32, tag="sc",
                                       name="scp", bufs=2)
                        for h2 in range(2):
                            nc.tensor.matmul(
                                scp[:, h2, :],
                                kT_sb[b][64 * h2:64 * h2 + 64,
                                         kc * 128:(kc + 1) * 128],
                                qT_sb[b][64 * h2:64 * h2 + 64,
                                        qb * 512:(qb + 1) * 512],
                                start=True, stop=True)
                        ex = expool.tile([128, 2, 512], BF16, tag="ex",
                                         name="ex", bufs=4)
                        nc.scalar.activation(ex[:], scp[:], AF.Exp,
                                             scale=0.125)
                        exs[kc] = ex
                        tick()
                        # Fill work lands here; av runs LAGGED two steps
                        # (its ex finished long ago) so the PE stream
                        # never waits on ACT's exp latency.
                        want = fcyc[0] // max(steps_rem[0], 1)
                        pop_budget(min(max(want, 1024), 4096))
                        steps_rem[0] -= 1
                        if kc >= 2:
                            do_av(kc - 2)
                    for t in (kcb - 2, kcb - 1):
                        if t >= 0:
                            pop_budget(1024)
                            do_av(t)
                    # Evacuate av right away (frees the PSUM slot for the
                    # next qb); the divide-by-denominator chain is deferred
                    # over the next qb's steps.
                    avc = wrk.tile([HD + 1, 2, 512], F32, tag="avc",
                                   name="avc")
                    # evacuate av on ACT (it has per-step slack and can
                    # read PSUM): keeps the qb-boundary DVE queue free
                    # for fill-slot CAST turnover
                    nc.scalar.activation(avc[:], av[:], AF.Copy)
                    r0 = wrk.tile([1, 1024], F32, tag="r0", name="r0",
                                  bufs=1)
                    nc.gpsimd.dma_start(r0[:], avc[HD:HD + 1, :, :])
                    cell = [None]
                    na = norm_a(avc, r0)
                    defer(1, (lambda cell=cell, na=na:
                              cell.__setitem__(0, na())))
                    defer(3, norm_b(b, qb, avc, cell))
                    if i == B - 1 and qb == 0:
                        # hold the last batch's first O-proj back as ready
                        # PE work for the drain, so the final normalize
                        # chain doesn't leave the PE idle
                        drain_reserve.append((b, qb))
                    else:
                        defer(6, (lambda bb=b, qq=qb: push_oproj(bb, qq)))
                    pop_budget(2048)
            for bb, qq in drain_reserve:
                push_oproj(bb, qq)
            while deferred or filler:
                if deferred:
                    deferred.sort(key=lambda x: x[0])
                    deferred.pop(0)[1]()
                pop_budget(2048)
            while filler:
                pop_one()


def _build(kcs):
    key = ("nc", tuple(kcs))
    if key in _cache:
        return _cache[key]
    nc = bacc.Bacc("TRN2", target_bir_lowering=False, debug=False,
                   num_devices=8)
    ap = {"wq": nc.dram_tensor("wq", [D, 128], BF16, kind="ExternalInput").ap(),
          "wk": nc.dram_tensor("wk", [D, 128], BF16, kind="ExternalInput").ap(),
          "wv": nc.dram_tensor("wv", [D, 2, HD], BF16, kind="ExternalInput").ap(),
          "wo": nc.dram_tensor("wo", [128, D], BF16, kind="ExternalInput").ap(),
          "vones": nc.dram_tensor("vones", [128, 64, 2], BF16,
                                  kind="ExternalInput").ap(),
          "out": nc.dram_tensor("out", [B, S, D], F16,
                                kind="ExternalOutput").ap()}
    for b in range(B):
        ap[f"xq{b}"] = nc.dram_tensor(f"xq{b}", [D, S], BF16,
                                      kind="ExternalInput").ap()
        ap[f"xk{b}"] = nc.dram_tensor(f"xk{b}", [D, kcs[b] * 128], BF16,
                                      kind="ExternalInput").ap()
        ap[f"xv{b}"] = nc.dram_tensor(f"xv{b}", [D, kcs[b] * 128], BF16,
                                      kind="ExternalInput").ap()
    with tile.TileContext(nc) as tc:
        _emit(nc, tc, ap, kcs)
    nc.compile()
    _cache[key] = nc
    return nc


def _in_maps(kcs, queries, keys, values, valid_len, W_q, W_k, W_v, W_o):
    bf = ml_dtypes.bfloat16
    # host-masked ones column: 1 where k < valid_len[b], else 0
    # vones[p, b*16+sc, h] = 1 if sc*128+p < valid_len[b] else 0
    kpos = np.arange(16 * 128).reshape(16, 128)
    vones = np.zeros((128, 64, 2), bf)
    for b in range(B):
        v1 = (kpos < int(valid_len[b])).astype(bf)  # [16, 128]
        vones[:, b * 16:(b + 1) * 16, :] = v1.T[:, :, None]
    maps = []
    for c in range(8):
        j0 = 128 * c
        m = {
            "wq": np.ascontiguousarray(W_q[j0:j0 + 128, :].T).astype(bf),
            "wk": np.ascontiguousarray(W_k[j0:j0 + 128, :].T).astype(bf),
            "wv": np.ascontiguousarray(
                W_v[j0:j0 + 128, :].T).astype(bf).reshape(D, 2, HD),
            "wo": np.ascontiguousarray(W_o[:, j0:j0 + 128].T).astype(bf),
            "vones": vones,
        }
        for b in range(B):
            nk = kcs[b] * 128
            xv = values[b][:nk].T.copy()      # [D, nk]
            xv[:, int(valid_len[b]):] = 0.0   # mask padding rows of v
            m[f"xq{b}"] = np.ascontiguousarray(queries[b].T).astype(bf)
            m[f"xk{b}"] = np.ascontiguousarray(keys[b][:nk].T).astype(bf)
            m[f"xv{b}"] = xv.astype(bf)
        maps.append(m)
    return maps


def kernel(queries, keys, values, valid_len, W_q, W_k, W_v, W_o,
           _run_kwargs=None):
    queries = np.asarray(queries, np.float32)
    keys = np.asarray(keys, np.float32)
    values = np.asarray(values, np.float32)
    valid_len = np.asarray(valid_len)
    W_q = np.asarray(W_q, np.float32)
    W_k = np.asarray(W_k, np.float32)
    W_v = np.asarray(W_v, np.float32)
    W_o = np.asarray(W_o, np.float32)

    kcs = [max(1, min(16, -(-int(valid_len[b]) // 128))) for b in range(B)]
    nc = _build(kcs)
    maps = _in_maps(kcs, queries, keys, values, valid_len, W_q, W_k, W_v, W_o)
    res = run_bass_kernel_spmd(nc, maps, list(range(8)), **(_run_kwargs or {}))
    out = np.zeros((B, S, D), np.float32)
    for c in range(8):
        out += res.results[c]["out"].astype(np.float32)
    if _run_kwargs:
        _cache["last_results"] = res
    return out
